# revision 38
# baseline (speedup 1.0000x reference)
"""Trainium2 Bass kernel for nn_NeuromorphicNetwork (8-core SPMD).

Math (same reduction as the original baseline): with REFRACT=1.0 and
current_time = spike_count, every neuron spikes AT MOST ONCE over the whole
batch scan and threshold adaptation never triggers.  Per neuron the batch
scan reduces to w_b = alpha*w_{b-1} + d_b (d = raw input current); the
spike-rate row is 0.1 at the first b with w_b >= THR (one-hot extracted via
prefix-max + shifted difference), else 0.

Implementation (241.8us baseline -> ~170us):
 - fp8e4m3 weights / counts / one-hot spikes + DoubleRow matmuls (K=256 per
   MM): GEMM1 is 128 MMs, GEMM2 32 MMs.  counts<=10 and fT in {0,1} are
   exact in fp8e4m3; weight rounding is ~1e-4 relative on the currents while
   first-crossing margins are orders of magnitude.
 - spike-count encode (stage A) entirely on DVE (concurrent DVE+GpSimd SBUF
   traffic degrades both ~2x), sigmoid on ScalarE; replication hoisted so
   compares start as soon as each uniform tile lands.
 - counts packed two-per-byte (c0 + 16*c1) and exchanged with two staged
   AllGathers (partition-major layout -> one contiguous DMA per rank on the
   way back; unpack via int8-rounding on ScalarE + one DVE op, scheduled in
   the PE-only chunk-A window).
 - GEMM1 chunk A (6 super-k, r-major) overlaps the second AllGather; chunk B
   is m-major with two held-back super-k per m-group, explicitly
   dependency-chained (the greedy scheduler otherwise transposes it to
   r-major and starves the scan pipeline).  Per-m first-crossing chains
   (PSUM-direct w-scan, prefix-max scan, gate + diff) run on DVE while PE
   continues; a GEMM2 wavefront fills PE gaps as psum banks retire.
 - output currents pre-scaled 1/64 into fp8, combined with AllToAll + local
   strided tree-sum (cheaper than ReduceScatter at this size), then the same
   first-crossing scan on the local 128-row output slice.
Cross-core skew + NRT barrier (~35-100us, run-variable) plus the
first-collective setup (~11us) dominate the remaining front of the kernel.
"""

import sys
import numpy as np

for _p in ("/opt/trn_rl_repo", "/root/.axon_site/_ro/trn_rl_repo"):
    if _p not in sys.path:
        sys.path.insert(0, _p)

import concourse.bass as bass
import concourse.mybir as mybir
import concourse.tile as tile
from concourse.tile_rust import add_dep_helper
from concourse import bacc
from concourse.bass_utils import run_bass_kernel_spmd

F32 = mybir.dt.float32
BF16 = mybir.dt.bfloat16
F8 = mybir.dt.float8e4
AL = mybir.AluOpType
ACT = mybir.ActivationFunctionType
DR = mybir.MatmulPerfMode.DoubleRow

B = 512            # batch (free dim everywhere)
IN_DIM = 4096
HID = 8192
OUT = 1024
T = 10
NCORES = 8
IN_SL = IN_DIM // NCORES    # 512 input dims per core
H_SL = HID // NCORES        # 1024 hidden per core
O_SL = OUT // NCORES        # 128 outputs per core
P = 128
SK = IN_DIM // 256          # 16 super-k (256-row) tiles for GEMM1
SM = H_SL // 256            # 4 super-m tiles for GEMM2

_LAM = np.float64(np.float32(0.95))
ALPHA = float(_LAM ** 10)                                # per-item decay
_G10 = float((1.0 - _LAM ** 10) / (1.0 - _LAM))          # per-item current gain
THR = float(10.0 / _G10)   # raw-current scan threshold (v>=1 <=> w>=THR)
RS_SCALE = 1.0 / 64.0      # pre-scale for fp8 output-current payload


def _build_nc():
    nc = bacc.Bacc(num_devices=NCORES)

    xt = nc.declare_dram_parameter("xt", [IN_SL, B], BF16, isOutput=False)
    u = nc.declare_dram_parameter("u", [IN_SL // P, P, T, B], BF16, isOutput=False)
    # w_ih[p, 2j+s, h] = W_ih[256j + 128s + p, hslice];  DoubleRow pairing
    w_ih = nc.declare_dram_parameter("w_ih", [P, 2 * SK, H_SL], F8, isOutput=False)
    # w_ho[p, 2j+s, o] = W_ho[hslice][256j + 128s + p, :]
    w_ho = nc.declare_dram_parameter("w_ho", [P, 2 * SM, OUT], F8, isOutput=False)
    res = nc.declare_dram_parameter("res", [O_SL, B], F32, isOutput=True)

    with tile.TileContext(nc, num_cores=NCORES) as tc:
        with (
            tc.tile_pool(name="const", bufs=1) as constp,
            tc.tile_pool(name="dram", bufs=1, space="DRAM") as dramp,
            tc.tile_pool(name="wih", bufs=1) as wpool,
            tc.tile_pool(name="who", bufs=1) as wopool,
            tc.tile_pool(name="stgA", bufs=2) as apool,
            tc.tile_pool(name="ubuf", bufs=2) as upool,
            tc.tile_pool(name="rep", bufs=2) as reppool,
            tc.tile_pool(name="cnt", bufs=2) as cntpool,
            tc.tile_pool(name="cp", bufs=8) as cppool,
            tc.tile_pool(name="scan", bufs=3) as spool,
            tc.tile_pool(name="fT", bufs=4) as fpool,
            tc.tile_pool(name="outb", bufs=4) as obpool,
            tc.tile_pool(name="fin", bufs=1) as finp,
        ):
            # ---- constants ----
            alpha_bf = constp.tile([P, B], BF16, name="alpha_bf")
            nc.vector.memset(alpha_bf, ALPHA)

            # ---- stage A: spike-count encode on the local input slice ----
            # counts land partition-major: cnt_localX[p, t*B + b] halves A/B.
            cnt_local = dramp.tile([P, 2 * B], mybir.dt.uint8, name="cnt_local")
            cnt_all = dramp.tile([NCORES * P, 2 * B], mybir.dt.uint8,
                                 name="cnt_all", addr_space="Shared")
            # hoist all input DMAs (sync queue is in-order; weights come later)
            xt_sbs, u_sbs = [], []
            for p in range(IN_SL // P):
                xt_sb = apool.tile([P, B], BF16, name="xt_sb", tag="xt", bufs=4)
                nc.sync.dma_start(xt_sb, xt[p * P:(p + 1) * P, :])
                xt_sbs.append(xt_sb)
            for p in range(IN_SL // P):
                u_sb = upool.tile([P, T * B], BF16, name="u_sb", tag="u", bufs=4)
                nc.sync.dma_start(u_sb, u[p].rearrange("p t b -> p (t b)"))
                u_sbs.append(u_sb)
            # sigmoids + replication first (no dependence on u loads)
            reps = []
            for p in range(IN_SL // P):
                sig = apool.tile([P, B], BF16, name="sig", tag="sig")
                nc.scalar.activation(sig, xt_sbs[p], ACT.Sigmoid)
                rep = reppool.tile([P, T * B], BF16, name="rep", tag="rep")
                nc.vector.tensor_copy(rep[:, 0:B], sig)
                nc.vector.tensor_copy(rep[:, B:2 * B], rep[:, 0:B])
                nc.vector.tensor_copy(rep[:, 2 * B:4 * B], rep[:, 0:2 * B])
                nc.vector.tensor_copy(rep[:, 4 * B:8 * B], rep[:, 0:4 * B])
                nc.vector.tensor_copy(rep[:, 8 * B:10 * B], rep[:, 0:2 * B])
                reps.append(rep)
            # compare + tree-sum as u tiles land (all on DVE: concurrent
            # DVE+GpSimd SBUF traffic degrades both ~2x)
            cnts = []
            for p in range(IN_SL // P):
                cmp = upool.tile([P, T * B], BF16, name="cmp", tag="cmp", bufs=1)
                nc.vector.tensor_tensor(cmp, u_sbs[p], reps[p], AL.is_lt)
                s1 = apool.tile([P, 5 * B], BF16, name="s1", tag="s1", bufs=1)
                nc.vector.tensor_tensor(s1, cmp[:, :5 * B], cmp[:, 5 * B:], AL.add)
                s2 = apool.tile([P, 2 * B], BF16, name="s2", tag="s2")
                nc.vector.tensor_tensor(s2, s1[:, :2 * B], s1[:, 2 * B:4 * B], AL.add)
                s3 = apool.tile([P, B], BF16, name="s3", tag="s3")
                nc.vector.tensor_tensor(s3, s2[:, :B], s2[:, B:], AL.add)
                cnt8 = cntpool.tile([P, B], BF16, name="cnt8", tag="cnt8")
                nc.vector.tensor_tensor(cnt8, s3, s1[:, 4 * B:], AL.add)
                cnts.append(cnt8)
                # pack two count tiles into one u8 plane: pk = c0 + 16*c1
                if p % 2 == 1:
                    pk = cntpool.tile([P, B], mybir.dt.uint8, name="pk", tag="pk")
                    nc.vector.scalar_tensor_tensor(
                        pk, cnts[p], 16.0, cnts[p - 1], AL.mult, AL.add)
                    nc.gpsimd.dma_start(
                        cnt_local[:, (p // 2) * B:(p // 2 + 1) * B], pk)
            nc.gpsimd.collective_compute(
                "AllGather", AL.bypass,
                replica_groups=[list(range(NCORES))],
                ins=[cnt_local[:, :]], outs=[cnt_all[:, :]],
            )

            # ---- weight loads (after stage-A DMAs; single large DMAs) ----
            w_sb = wpool.tile([P, 2 * SK, H_SL], F8, name="w_sb")
            nc.sync.dma_start(
                w_sb.rearrange("p j h -> p (j h)"),
                w_ih.rearrange("p j h -> p (j h)"),
            )
            who_sb = wopool.tile([P, 2 * SM, OUT], F8, name="who_sb")
            nc.sync.dma_start(
                who_sb.rearrange("p j o -> p (j o)"),
                w_ho.rearrange("p j o -> p (j o)"),
            )

            with tc.tile_pool(name="ps", bufs=8, space="PSUM") as psp:
                psum_h = [psp.tile([P, B], F32, name=f"ph{m}", tag="ph")
                          for m in range(H_SL // P)]

                # per-rank packed counts -> cp[r] [P, 4, B] fp8:
                # tiles (0,1)=(c0,c1) of plane A (super-k 2r), (2,3) of B (2r+1)
                cps = []
                for r in range(NCORES):
                    pk = cppool.tile([P, 2 * B], mybir.dt.uint8, name=f"pk{r}",
                                     tag="pk", bufs=3)
                    nc.sync.dma_start(pk, cnt_all[r * P:(r + 1) * P, :])
                    cp = cppool.tile([P, 4, B], F8, name=f"cp{r}", tag="cp",
                                     bufs=8)
                    cpf = cp.rearrange("p s b -> p (s b)")
                    for h in range(2):
                        nc.scalar.activation(
                            cpf[:, (2 * h + 1) * B:(2 * h + 2) * B],
                            pk[:, h * B:(h + 1) * B], ACT.Copy,
                            scale=1.0 / 16.0, bias=-5.0 / 16.0)
                        nc.vector.scalar_tensor_tensor(
                            cpf[:, 2 * h * B:(2 * h + 1) * B],
                            cpf[:, (2 * h + 1) * B:(2 * h + 2) * B], -16.0,
                            pk[:, h * B:(h + 1) * B], AL.mult, AL.add)
                    cps.append(cp)

                fT = []        # [P, 2, B] fp8 one-hot spike tiles per super-m
                a2a_in = dramp.tile([OUT, B], F8, name="a2a_in")
                psum_o = [psp.tile([P, B], F32, name=f"po{o}", tag="ph")
                          for o in range(OUT // P)]
                g2_emitted = set()

                def emit_gemm2(avail_o, max_j):
                    for jj in range(max_j + 1):
                        for o in range(avail_o):
                            if (jj, o) in g2_emitted:
                                continue
                            g2_emitted.add((jj, o))
                            nc.tensor.matmul(
                                psum_o[o],
                                lhsT=who_sb[:, 2 * jj:2 * jj + 2,
                                            o * P:(o + 1) * P],
                                rhs=fT[jj][:, :, :],
                                start=(jj == 0), stop=(jj == SM - 1),
                                perf_mode=DR,
                            )
                            if jj == SM - 1:
                                ob = obpool.tile([P, B], F8, name="ob", tag="ob")
                                if o % 2 == 0:
                                    nc.scalar.activation(ob, psum_o[o], ACT.Copy,
                                                         scale=RS_SCALE)
                                else:
                                    nc.vector.tensor_scalar(
                                        ob, psum_o[o], RS_SCALE, None, AL.mult)
                                (nc.sync if o % 2 == 0 else nc.scalar).dma_start(
                                    a2a_in[o * P:(o + 1) * P, :], ob)

                f8t = None
                pm_pair = None
                prev_last_mm = None
                for m in range(H_SL // P):
                    first_mm = None
                    for r in range(NCORES):
                        for h in range(2):
                            j = 2 * r + h
                            mm = nc.tensor.matmul(
                                psum_h[m],
                                lhsT=w_sb[:, 2 * j:2 * j + 2, m * P:(m + 1) * P],
                                rhs=cps[r][:, 2 * h:2 * h + 2, :],
                                start=(r == 0 and h == 0),
                                stop=(r == NCORES - 1 and h == 1),
                                perf_mode=DR,
                            )
                            first_mm = first_mm or mm
                    if prev_last_mm is not None:
                        # pin m-major order (greedy scheduler otherwise
                        # transposes to r-major and starves the scans)
                        add_dep_helper(first_mm.ins, prev_last_mm.ins, sync=True,
                                       reason="m-major order")
                    prev_last_mm = mm
                    # first-crossing chain for m-tile (overlaps later matmuls):
                    # scans on DVE (only engine with scan), gate+diff on GpSimd.
                    # the w-scan reads PSUM directly (same 1x speed, frees the
                    # bank on completion, no Scalar evac in the chain)
                    w_sc = spool.tile([P, B], BF16, name="w_sc", tag="wsc")
                    nc.vector.tensor_tensor_scan(
                        w_sc, alpha_bf, psum_h[m], 0.0, AL.mult, AL.add)
                    if m % 2 == 0:
                        pm_pair = spool.tile([P, 2 * B], BF16, name="pm", tag="pm")
                        f8t = fpool.tile([P, 2, B], F8, name=f"fT{m // 2}", tag="fT")
                    nc.vector.tensor_tensor_scan(
                        pm_pair[:, (m % 2) * B:(m % 2 + 1) * B],
                        w_sc, w_sc, 0.0, AL.max, AL.max)
                    if m % 2 == 1:
                        # gate + step-up diff on DVE: shortest fT chain; the
                        # GEMM2 wavefront keeps PE ahead of DVE throughput
                        f8f = f8t.rearrange("p s b -> p (s b)")
                        gp = spool.tile([P, 2 * B], BF16, name="gp", tag="gp")
                        nc.vector.tensor_scalar(gp, pm_pair, THR, None, AL.is_ge)
                        nc.vector.tensor_tensor(
                            f8f[:, 1:], gp[:, 1:], gp[:, :2 * B - 1], AL.subtract)
                        nc.scalar.copy(f8f[:, 0:1], gp[:, 0:1])
                        nc.scalar.copy(f8f[:, B:B + 1], gp[:, B:B + 1])
                        fT.append(f8t)
                        # GEMM2 wavefront: emit every (j, o) whose fT and psum
                        # bank are now available, filling PE's scan-wait gaps
                        emit_gemm2(avail_o=min(2 * (m // 2 + 1), OUT // P),
                                   max_j=m // 2)

            # ---- AllToAll: shard o-slices across cores; local tree-sum ----
            a2a_out = dramp.tile([OUT, B], F8, name="a2a_out")
            nc.gpsimd.collective_compute(
                "AllToAll", AL.bypass,
                replica_groups=[list(range(NCORES))],
                ins=[a2a_in[:, :]], outs=[a2a_out[:, :]],
            )
            # load the 8 partial tiles into one [P, 8, B] buffer (issue spread
            # across the three DMA-capable queues), then 3 strided tree adds
            pbuf = finp.tile([P, NCORES, B], F8, name="pbuf")
            pflat = pbuf.rearrange("p i b -> p (i b)")
            engs = [nc.sync, nc.scalar]   # gpsimd queue pays a DRAIN here
            for i in range(NCORES):
                engs[i % 2].dma_start(pflat[:, i * B:(i + 1) * B],
                                      a2a_out[i * P:(i + 1) * P, :])
            q4 = finp.tile([P, 4, B], BF16, name="q4")
            nc.vector.tensor_tensor(q4[:, :, :], pbuf[:, 0:4, :],
                                    pbuf[:, 4:8, :], AL.add)
            q2 = finp.tile([P, 2, B], BF16, name="q2")
            nc.vector.tensor_tensor(q2[:, :, :], q4[:, 0:2, :],
                                    q4[:, 2:4, :], AL.add)
            cur_o = finp.tile([P, B], BF16, name="cur_o")
            nc.vector.tensor_tensor(cur_o, q2[:, 0, :], q2[:, 1, :], AL.add)

            # ---- output layer: same scan + first-crossing, scaled by 0.1 ----
            wo = spool.tile([P, B], BF16, name="wo", tag="wsc")
            nc.vector.tensor_tensor_scan(wo, alpha_bf, cur_o, 0.0, AL.mult, AL.add)
            pmo = spool.tile([P, B], BF16, name="pmo", tag="pm2")
            nc.vector.tensor_tensor_scan(pmo, wo, wo, 0.0, AL.max, AL.max)
            g2 = spool.tile([P, B], F32, name="g2", tag="g2")
            nc.vector.tensor_scalar(
                g2, pmo, THR * RS_SCALE, float(np.float32(0.1)),
                AL.is_ge, AL.mult)
            out_sb = spool.tile([P, B], F32, name="out_sb", tag="outsb")
            nc.vector.tensor_tensor(
                out_sb[:, 1:], g2[:, 1:], g2[:, :B - 1], AL.subtract)
            nc.vector.tensor_copy(out_sb[:, 0:1], g2[:, 0:1])
            nc.sync.dma_start(res[:, :], out_sb)

    nc.finalize()
    return nc


_STATE = {}


def _get_uniforms():
    """The key-42 uniform draws the reference's bernoulli uses — input-independent
    constants. [B, IN_DIM, T] float32, computed once on host."""
    if "u" not in _STATE:
        import jax
        import jax.numpy as jnp
        f = jax.jit(lambda: jax.random.uniform(
            jax.random.key(42), (B, IN_DIM, T), jnp.float32), backend="cpu")
        _STATE["u"] = np.asarray(f())
    return _STATE["u"]


def _get_nc():
    if "nc" not in _STATE:
        _STATE["nc"] = _build_nc()
    return _STATE["nc"]


def make_in_maps(x, W_ih, W_ho):
    import ml_dtypes

    F8NP = ml_dtypes.float8_e4m3
    x = np.ascontiguousarray(x, dtype=np.float32)
    W_ih = np.ascontiguousarray(W_ih, dtype=np.float32)
    W_ho = np.ascontiguousarray(W_ho, dtype=np.float32)
    u = _get_uniforms()

    in_maps = []
    for m in range(NCORES):
        isl = slice(m * IN_SL, (m + 1) * IN_SL)
        # u[b, i, t] -> [i_slice, t, b] -> [4, 128, T, B] bf16
        uc = np.ascontiguousarray(
            u[:, isl, :].transpose(1, 2, 0).reshape(IN_SL // P, P, T, B)
        ).astype(ml_dtypes.bfloat16)
        # W_ih[:, hslice] -> [p, 2j+s, h] DoubleRow pairing, fp8
        wi = W_ih[:, m * H_SL:(m + 1) * H_SL].reshape(SK, 2, P, H_SL)
        wi = np.ascontiguousarray(wi.transpose(2, 0, 1, 3).reshape(P, 2 * SK, H_SL))
        # W_ho[hslice, :] -> [p, 2j+s, o], fp8
        wo = W_ho[m * H_SL:(m + 1) * H_SL, :].reshape(SM, 2, P, OUT)
        wo = np.ascontiguousarray(wo.transpose(2, 0, 1, 3).reshape(P, 2 * SM, OUT))
        in_maps.append({
            "xt": np.ascontiguousarray(x[:, isl].T).astype(ml_dtypes.bfloat16),
            "u": uc,
            "w_ih": wi.astype(F8NP),
            "w_ho": wo.astype(F8NP),
        })
    return in_maps


def assemble_out(results):
    out = np.empty((B, OUT), np.float32)
    for m in range(NCORES):
        out[:, m * O_SL:(m + 1) * O_SL] = results[m]["res"].T
    return out


def kernel(x, W_ih, W_ho):
    nc = _get_nc()
    in_maps = make_in_maps(x, W_ih, W_ho)
    r = run_bass_kernel_spmd(nc, in_maps, list(range(NCORES)))

    return assemble_out(r.results)


if __name__ == "__main__":
    rng = np.random.default_rng(0)
    x = rng.standard_normal((B, IN_DIM), dtype=np.float32)
    W_ih = np.clip(0.5 + 0.1 * rng.standard_normal((IN_DIM, HID)), 0, 1).astype(np.float32)
    W_ho = np.clip(0.5 + 0.1 * rng.standard_normal((HID, OUT)), 0, 1).astype(np.float32)
    out = kernel(x, W_ih, W_ho)
    print("out", out.shape, out.dtype, "nonzero rows:", np.unique(np.nonzero(out)[0]))


# revision 39
# speedup vs baseline: 1.0828x; 1.0828x over previous
"""Trainium2 Bass kernel for nn_NeuromorphicNetwork (8-core SPMD).

Math (same reduction as the original baseline): with REFRACT=1.0 and
current_time = spike_count, every neuron spikes AT MOST ONCE over the whole
batch scan and threshold adaptation never triggers.  Per neuron the batch
scan reduces to w_b = alpha*w_{b-1} + d_b (d = raw input current); the
spike-rate row is 0.1 at the first b with w_b >= THR (one-hot extracted via
prefix-max + shifted difference), else 0.

Implementation (241.8us baseline -> ~170us):
 - fp8e4m3 weights / counts / one-hot spikes + DoubleRow matmuls (K=256 per
   MM): GEMM1 is 128 MMs, GEMM2 32 MMs.  counts<=10 and fT in {0,1} are
   exact in fp8e4m3; weight rounding is ~1e-4 relative on the currents while
   first-crossing margins are orders of magnitude.
 - spike-count encode (stage A) entirely on DVE (concurrent DVE+GpSimd SBUF
   traffic degrades both ~2x), sigmoid on ScalarE; replication hoisted so
   compares start as soon as each uniform tile lands.
 - counts packed two-per-byte (c0 + 16*c1) and exchanged with two staged
   AllGathers (partition-major layout -> one contiguous DMA per rank on the
   way back; unpack via int8-rounding on ScalarE + one DVE op, scheduled in
   the PE-only chunk-A window).
 - GEMM1 chunk A (6 super-k, r-major) overlaps the second AllGather; chunk B
   is m-major with two held-back super-k per m-group, explicitly
   dependency-chained (the greedy scheduler otherwise transposes it to
   r-major and starves the scan pipeline).  Per-m first-crossing chains
   (PSUM-direct w-scan, prefix-max scan, gate + diff) run on DVE while PE
   continues; a GEMM2 wavefront fills PE gaps as psum banks retire.
 - output currents pre-scaled 1/64 into fp8, combined with AllToAll + local
   strided tree-sum (cheaper than ReduceScatter at this size), then the same
   first-crossing scan on the local 128-row output slice.
Cross-core skew + NRT barrier (~35-100us, run-variable) plus the
first-collective setup (~11us) dominate the remaining front of the kernel.
"""

import sys
import numpy as np

for _p in ("/opt/trn_rl_repo", "/root/.axon_site/_ro/trn_rl_repo"):
    if _p not in sys.path:
        sys.path.insert(0, _p)

import concourse.bass as bass
import concourse.mybir as mybir
import concourse.tile as tile
from concourse.tile_rust import add_dep_helper
from concourse import bacc
from concourse.bass_utils import run_bass_kernel_spmd

F32 = mybir.dt.float32
BF16 = mybir.dt.bfloat16
F8 = mybir.dt.float8e4
AL = mybir.AluOpType
ACT = mybir.ActivationFunctionType
DR = mybir.MatmulPerfMode.DoubleRow

B = 512            # batch (free dim everywhere)
IN_DIM = 4096
HID = 8192
OUT = 1024
T = 10
NCORES = 8
IN_SL = IN_DIM // NCORES    # 512 input dims per core
H_SL = HID // NCORES        # 1024 hidden per core
O_SL = OUT // NCORES        # 128 outputs per core
P = 128
SK = IN_DIM // 256          # 16 super-k (256-row) tiles for GEMM1
SM = H_SL // 256            # 4 super-m tiles for GEMM2

_LAM = np.float64(np.float32(0.95))
ALPHA = float(_LAM ** 10)                                # per-item decay
_G10 = float((1.0 - _LAM ** 10) / (1.0 - _LAM))          # per-item current gain
THR = float(10.0 / _G10)   # raw-current scan threshold (v>=1 <=> w>=THR)
RS_SCALE = 1.0 / 64.0      # pre-scale for fp8 output-current payload


def _build_nc():
    nc = bacc.Bacc(num_devices=NCORES)

    xt = nc.declare_dram_parameter("xt", [IN_SL, B], BF16, isOutput=False)
    u = nc.declare_dram_parameter("u", [IN_SL // P, P, T, B], BF16, isOutput=False)
    # w_ih[p, 2j+s, h] = W_ih[256j + 128s + p, hslice];  DoubleRow pairing
    w_ih = nc.declare_dram_parameter("w_ih", [P, 2 * SK, H_SL], F8, isOutput=False)
    # w_ho[p, 2j+s, o] = W_ho[hslice][256j + 128s + p, :]
    w_ho = nc.declare_dram_parameter("w_ho", [P, 2 * SM, OUT], F8, isOutput=False)
    res = nc.declare_dram_parameter("res", [O_SL, B], F32, isOutput=True)

    with tile.TileContext(nc, num_cores=NCORES) as tc:
        with (
            tc.tile_pool(name="const", bufs=1) as constp,
            tc.tile_pool(name="dram", bufs=1, space="DRAM") as dramp,
            tc.tile_pool(name="wih", bufs=1) as wpool,
            tc.tile_pool(name="who", bufs=1) as wopool,
            tc.tile_pool(name="stgA", bufs=2) as apool,
            tc.tile_pool(name="ubuf", bufs=2) as upool,
            tc.tile_pool(name="rep", bufs=2) as reppool,
            tc.tile_pool(name="cnt", bufs=2) as cntpool,
            tc.tile_pool(name="cp", bufs=8) as cppool,
            tc.tile_pool(name="scan", bufs=3) as spool,
            tc.tile_pool(name="fT", bufs=4) as fpool,
            tc.tile_pool(name="outb", bufs=4) as obpool,
            tc.tile_pool(name="fin", bufs=1) as finp,
        ):
            # ---- constants ----
            alpha_bf = constp.tile([P, B], BF16, name="alpha_bf")
            nc.vector.memset(alpha_bf, ALPHA)

            # ---- stage A: spike-count encode on the local input slice ----
            # counts land partition-major: cnt_localX[p, t*B + b] halves A/B.
            cnt_localA = dramp.tile([P, B], mybir.dt.uint8, name="cnt_localA")
            cnt_localB = dramp.tile([P, B], mybir.dt.uint8, name="cnt_localB")
            cnt_allA = dramp.tile([NCORES * P, B], mybir.dt.uint8, name="cnt_allA",
                                  addr_space="Shared")
            cnt_allB = dramp.tile([NCORES * P, B], mybir.dt.uint8, name="cnt_allB",
                                  addr_space="Shared")
            # hoist all input DMAs (sync queue is in-order; weights come later)
            xt_sbs, u_sbs = [], []
            for p in range(IN_SL // P):
                xt_sb = apool.tile([P, B], BF16, name="xt_sb", tag="xt", bufs=4)
                nc.sync.dma_start(xt_sb, xt[p * P:(p + 1) * P, :])
                xt_sbs.append(xt_sb)
            for p in range(IN_SL // P):
                u_sb = upool.tile([P, T * B], BF16, name="u_sb", tag="u", bufs=4)
                nc.sync.dma_start(u_sb, u[p].rearrange("p t b -> p (t b)"))
                u_sbs.append(u_sb)
            # sigmoids + replication first (no dependence on u loads)
            reps = []
            for p in range(IN_SL // P):
                sig = apool.tile([P, B], BF16, name="sig", tag="sig")
                nc.scalar.activation(sig, xt_sbs[p], ACT.Sigmoid)
                rep = reppool.tile([P, T * B], BF16, name="rep", tag="rep")
                nc.vector.tensor_copy(rep[:, 0:B], sig)
                nc.vector.tensor_copy(rep[:, B:2 * B], rep[:, 0:B])
                nc.vector.tensor_copy(rep[:, 2 * B:4 * B], rep[:, 0:2 * B])
                nc.vector.tensor_copy(rep[:, 4 * B:8 * B], rep[:, 0:4 * B])
                nc.vector.tensor_copy(rep[:, 8 * B:10 * B], rep[:, 0:2 * B])
                reps.append(rep)
            # compare + tree-sum as u tiles land (all on DVE: concurrent
            # DVE+GpSimd SBUF traffic degrades both ~2x)
            cnts = []
            for p in range(IN_SL // P):
                cmp = upool.tile([P, T * B], BF16, name="cmp", tag="cmp", bufs=1)
                nc.vector.tensor_tensor(cmp, u_sbs[p], reps[p], AL.is_lt)
                s1 = apool.tile([P, 5 * B], BF16, name="s1", tag="s1", bufs=1)
                nc.vector.tensor_tensor(s1, cmp[:, :5 * B], cmp[:, 5 * B:], AL.add)
                s2 = apool.tile([P, 2 * B], BF16, name="s2", tag="s2")
                nc.vector.tensor_tensor(s2, s1[:, :2 * B], s1[:, 2 * B:4 * B], AL.add)
                s3 = apool.tile([P, B], BF16, name="s3", tag="s3")
                nc.vector.tensor_tensor(s3, s2[:, :B], s2[:, B:], AL.add)
                cnt8 = cntpool.tile([P, B], BF16, name="cnt8", tag="cnt8")
                nc.vector.tensor_tensor(cnt8, s3, s1[:, 4 * B:], AL.add)
                cnts.append(cnt8)
                # pack two count tiles into one u8 plane: pk = c0 + 16*c1
                if p % 2 == 1:
                    pk = cntpool.tile([P, B], mybir.dt.uint8, name="pk", tag="pk")
                    nc.vector.scalar_tensor_tensor(
                        pk, cnts[p], 16.0, cnts[p - 1], AL.mult, AL.add)
                    dst = cnt_localA if p < 2 else cnt_localB
                    nc.gpsimd.dma_start(dst[:, :], pk)
                    nc.gpsimd.collective_compute(
                        "AllGather", AL.bypass,
                        replica_groups=[list(range(NCORES))],
                        ins=[(cnt_localA if p < 2 else cnt_localB)[:, :]],
                        outs=[(cnt_allA if p < 2 else cnt_allB)[:, :]],
                    )

            # ---- weight loads (after stage-A DMAs; single large DMAs) ----
            w_sb = wpool.tile([P, 2 * SK, H_SL], F8, name="w_sb")
            nc.sync.dma_start(
                w_sb.rearrange("p j h -> p (j h)"),
                w_ih.rearrange("p j h -> p (j h)"),
            )
            who_sb = wopool.tile([P, 2 * SM, OUT], F8, name="who_sb")
            nc.sync.dma_start(
                who_sb.rearrange("p j o -> p (j o)"),
                w_ho.rearrange("p j o -> p (j o)"),
            )

            with tc.tile_pool(name="ps", bufs=8, space="PSUM") as psp:
                psum_h = [psp.tile([P, B], F32, name=f"ph{m}", tag="ph")
                          for m in range(H_SL // P)]

                # ---- GEMM1 chunk A (r-major, super-k 2r; overlaps AG-B) ----
                def load_unpack(r, cnt_allX, tag, bufs):
                    pk = cppool.tile([P, B], mybir.dt.uint8, name=f"pk{tag}{r}",
                                     tag=f"pk{tag}", bufs=3)
                    nc.sync.dma_start(pk, cnt_allX[r * P:(r + 1) * P, :])
                    cp = cppool.tile([P, 2, B], F8, name=f"cp{tag}{r}",
                                     tag=f"cp{tag}", bufs=bufs)
                    cpf = cp.rearrange("p s b -> p (s b)")
                    # c1 ~= RNE_f8(pk/16 - 5/16) on ScalarE, c0 = pk - 16*c1 on
                    # DVE.  The f8 rounding is exact to the integer for c1>=4;
                    # below that c1 keeps a bounded fraction (|err|<=0.31) that
                    # c0 absorbs with the opposite sign, so the reconstructed
                    # current error per element is zero-mean and ~1e-4 relative
                    # after the 4096-wide contraction — harmless against the
                    # first-crossing margins (verified bit-exact on HW).
                    nc.scalar.activation(cpf[:, B:2 * B], pk, ACT.Copy,
                                         scale=1.0 / 16.0, bias=-5.0 / 16.0)
                    nc.vector.scalar_tensor_tensor(
                        cpf[:, 0:B], cpf[:, B:2 * B], -16.0, pk, AL.mult, AL.add)
                    return cp

                HOLD = 2      # super-k held back to feed PE during chunk B
                cpa_held = []
                for r in range(NCORES):
                    cp = load_unpack(r, cnt_allA, "a", 3)
                    if r >= NCORES - HOLD:
                        cpa_held.append((2 * r, cp))
                        continue
                    j = 2 * r
                    for m in range(H_SL // P):
                        nc.tensor.matmul(
                            psum_h[m],
                            lhsT=w_sb[:, 2 * j:2 * j + 2, m * P:(m + 1) * P],
                            rhs=cp[:, :, :],
                            start=(r == 0), stop=False,
                            perf_mode=DR,
                        )

                # ---- GEMM1 chunk B (m-major, super-k 2r+1) + pipelined scans --
                cpb = []
                for r in range(NCORES):
                    cpb.append(load_unpack(r, cnt_allB, "b", 8))

                fT = []        # [P, 2, B] fp8 one-hot spike tiles per super-m
                a2a_in = dramp.tile([OUT, B], F8, name="a2a_in")
                psum_o = [psp.tile([P, B], F32, name=f"po{o}", tag="ph")
                          for o in range(OUT // P)]
                g2_emitted = set()

                def emit_gemm2(avail_o, max_j):
                    for jj in range(max_j + 1):
                        for o in range(avail_o):
                            if (jj, o) in g2_emitted:
                                continue
                            g2_emitted.add((jj, o))
                            nc.tensor.matmul(
                                psum_o[o],
                                lhsT=who_sb[:, 2 * jj:2 * jj + 2,
                                            o * P:(o + 1) * P],
                                rhs=fT[jj][:, :, :],
                                start=(jj == 0), stop=(jj == SM - 1),
                                perf_mode=DR,
                            )
                            if jj == SM - 1:
                                ob = obpool.tile([P, B], F8, name="ob", tag="ob")
                                if o % 2 == 0:
                                    nc.scalar.activation(ob, psum_o[o], ACT.Copy,
                                                         scale=RS_SCALE)
                                else:
                                    nc.vector.tensor_scalar(
                                        ob, psum_o[o], RS_SCALE, None, AL.mult)
                                (nc.sync if o % 2 == 0 else nc.scalar).dma_start(
                                    a2a_in[o * P:(o + 1) * P, :], ob)

                f8t = None
                pm_pair = None
                prev_last_mm = None
                for m in range(H_SL // P):
                    first_mm = None
                    for j, cp in cpa_held:
                        mm = nc.tensor.matmul(
                            psum_h[m],
                            lhsT=w_sb[:, 2 * j:2 * j + 2, m * P:(m + 1) * P],
                            rhs=cp[:, :, :],
                            start=False, stop=False,
                            perf_mode=DR,
                        )
                        first_mm = first_mm or mm
                    for r in range(NCORES):
                        j = 2 * r + 1
                        mm = nc.tensor.matmul(
                            psum_h[m],
                            lhsT=w_sb[:, 2 * j:2 * j + 2, m * P:(m + 1) * P],
                            rhs=cpb[r][:, :, :],
                            start=False, stop=(r == NCORES - 1),
                            perf_mode=DR,
                        )
                        first_mm = first_mm or mm
                    if prev_last_mm is not None:
                        # pin m-major order: the greedy scheduler otherwise
                        # transposes chunk B to r-major (cpb tiles become
                        # ready progressively), starving the scan pipeline
                        add_dep_helper(first_mm.ins, prev_last_mm.ins, sync=True,
                                       reason="chunk-B m-major order")
                    prev_last_mm = mm
                    # first-crossing chain for m-tile (overlaps later matmuls):
                    # scans on DVE (only engine with scan), gate+diff on GpSimd.
                    # the w-scan reads PSUM directly (same 1x speed, frees the
                    # bank on completion, no Scalar evac in the chain)
                    w_sc = spool.tile([P, B], BF16, name="w_sc", tag="wsc")
                    nc.vector.tensor_tensor_scan(
                        w_sc, alpha_bf, psum_h[m], 0.0, AL.mult, AL.add)
                    if m % 2 == 0:
                        pm_pair = spool.tile([P, 2 * B], BF16, name="pm", tag="pm")
                        f8t = fpool.tile([P, 2, B], F8, name=f"fT{m // 2}", tag="fT")
                    nc.vector.tensor_tensor_scan(
                        pm_pair[:, (m % 2) * B:(m % 2 + 1) * B],
                        w_sc, w_sc, 0.0, AL.max, AL.max)
                    if m % 2 == 1:
                        # gate + step-up diff on DVE: shortest fT chain; the
                        # GEMM2 wavefront keeps PE ahead of DVE throughput
                        f8f = f8t.rearrange("p s b -> p (s b)")
                        gp = spool.tile([P, 2 * B], BF16, name="gp", tag="gp")
                        nc.vector.tensor_scalar(gp, pm_pair, THR, None, AL.is_ge)
                        nc.vector.tensor_tensor(
                            f8f[:, 1:], gp[:, 1:], gp[:, :2 * B - 1], AL.subtract)
                        nc.scalar.copy(f8f[:, 0:1], gp[:, 0:1])
                        nc.scalar.copy(f8f[:, B:B + 1], gp[:, B:B + 1])
                        fT.append(f8t)
                        # GEMM2 wavefront: emit every (j, o) whose fT and psum
                        # bank are now available, filling PE's scan-wait gaps
                        emit_gemm2(avail_o=min(2 * (m // 2 + 1), OUT // P),
                                   max_j=m // 2)

            # ---- AllToAll: shard o-slices across cores; local tree-sum ----
            a2a_out = dramp.tile([OUT, B], F8, name="a2a_out")
            nc.gpsimd.collective_compute(
                "AllToAll", AL.bypass,
                replica_groups=[list(range(NCORES))],
                ins=[a2a_in[:, :]], outs=[a2a_out[:, :]],
            )
            # load the 8 partial tiles into one [P, 8, B] buffer (issue spread
            # across the three DMA-capable queues), then 3 strided tree adds
            pbuf = finp.tile([P, NCORES, B], F8, name="pbuf")
            pflat = pbuf.rearrange("p i b -> p (i b)")
            engs = [nc.sync, nc.scalar]   # gpsimd queue pays a DRAIN here
            for i in range(NCORES):
                engs[i % 2].dma_start(pflat[:, i * B:(i + 1) * B],
                                      a2a_out[i * P:(i + 1) * P, :])
            q4 = finp.tile([P, 4, B], BF16, name="q4")
            nc.vector.tensor_tensor(q4[:, :, :], pbuf[:, 0:4, :],
                                    pbuf[:, 4:8, :], AL.add)
            q2 = finp.tile([P, 2, B], BF16, name="q2")
            nc.vector.tensor_tensor(q2[:, :, :], q4[:, 0:2, :],
                                    q4[:, 2:4, :], AL.add)
            cur_o = finp.tile([P, B], BF16, name="cur_o")
            nc.vector.tensor_tensor(cur_o, q2[:, 0, :], q2[:, 1, :], AL.add)

            # ---- output layer: same scan + first-crossing, scaled by 0.1 ----
            wo = spool.tile([P, B], BF16, name="wo", tag="wsc")
            nc.vector.tensor_tensor_scan(wo, alpha_bf, cur_o, 0.0, AL.mult, AL.add)
            pmo = spool.tile([P, B], BF16, name="pmo", tag="pm2")
            nc.vector.tensor_tensor_scan(pmo, wo, wo, 0.0, AL.max, AL.max)
            g2 = spool.tile([P, B], F32, name="g2", tag="g2")
            nc.vector.tensor_scalar(
                g2, pmo, THR * RS_SCALE, float(np.float32(0.1)),
                AL.is_ge, AL.mult)
            out_sb = spool.tile([P, B], F32, name="out_sb", tag="outsb")
            nc.vector.tensor_tensor(
                out_sb[:, 1:], g2[:, 1:], g2[:, :B - 1], AL.subtract)
            nc.vector.tensor_copy(out_sb[:, 0:1], g2[:, 0:1])
            nc.sync.dma_start(res[:, :], out_sb)

    nc.finalize()
    return nc


_STATE = {}


def _get_uniforms():
    """The key-42 uniform draws the reference's bernoulli uses — input-independent
    constants. [B, IN_DIM, T] float32, computed once on host."""
    if "u" not in _STATE:
        import jax
        import jax.numpy as jnp
        f = jax.jit(lambda: jax.random.uniform(
            jax.random.key(42), (B, IN_DIM, T), jnp.float32), backend="cpu")
        _STATE["u"] = np.asarray(f())
    return _STATE["u"]


def _get_nc():
    if "nc" not in _STATE:
        _STATE["nc"] = _build_nc()
    return _STATE["nc"]


def make_in_maps(x, W_ih, W_ho):
    import ml_dtypes

    F8NP = ml_dtypes.float8_e4m3
    x = np.ascontiguousarray(x, dtype=np.float32)
    W_ih = np.ascontiguousarray(W_ih, dtype=np.float32)
    W_ho = np.ascontiguousarray(W_ho, dtype=np.float32)
    u = _get_uniforms()

    in_maps = []
    for m in range(NCORES):
        isl = slice(m * IN_SL, (m + 1) * IN_SL)
        # u[b, i, t] -> [i_slice, t, b] -> [4, 128, T, B] bf16
        uc = np.ascontiguousarray(
            u[:, isl, :].transpose(1, 2, 0).reshape(IN_SL // P, P, T, B)
        ).astype(ml_dtypes.bfloat16)
        # W_ih[:, hslice] -> [p, 2j+s, h] DoubleRow pairing, fp8
        wi = W_ih[:, m * H_SL:(m + 1) * H_SL].reshape(SK, 2, P, H_SL)
        wi = np.ascontiguousarray(wi.transpose(2, 0, 1, 3).reshape(P, 2 * SK, H_SL))
        # W_ho[hslice, :] -> [p, 2j+s, o], fp8
        wo = W_ho[m * H_SL:(m + 1) * H_SL, :].reshape(SM, 2, P, OUT)
        wo = np.ascontiguousarray(wo.transpose(2, 0, 1, 3).reshape(P, 2 * SM, OUT))
        in_maps.append({
            "xt": np.ascontiguousarray(x[:, isl].T).astype(ml_dtypes.bfloat16),
            "u": uc,
            "w_ih": wi.astype(F8NP),
            "w_ho": wo.astype(F8NP),
        })
    return in_maps


def assemble_out(results):
    out = np.empty((B, OUT), np.float32)
    for m in range(NCORES):
        out[:, m * O_SL:(m + 1) * O_SL] = results[m]["res"].T
    return out


def kernel(x, W_ih, W_ho):
    nc = _get_nc()
    in_maps = make_in_maps(x, W_ih, W_ho)
    r = run_bass_kernel_spmd(nc, in_maps, list(range(NCORES)))

    return assemble_out(r.results)


if __name__ == "__main__":
    rng = np.random.default_rng(0)
    x = rng.standard_normal((B, IN_DIM), dtype=np.float32)
    W_ih = np.clip(0.5 + 0.1 * rng.standard_normal((IN_DIM, HID)), 0, 1).astype(np.float32)
    W_ho = np.clip(0.5 + 0.1 * rng.standard_normal((HID, OUT)), 0, 1).astype(np.float32)
    out = kernel(x, W_ih, W_ho)
    print("out", out.shape, out.dtype, "nonzero rows:", np.unique(np.nonzero(out)[0]))


# revision 40
# speedup vs baseline: 1.1644x; 1.0753x over previous
"""Trainium2 Bass kernel for nn_NeuromorphicNetwork (8-core SPMD).

Math (same reduction as the original baseline): with REFRACT=1.0 and
current_time = spike_count, every neuron spikes AT MOST ONCE over the whole
batch scan and threshold adaptation never triggers.  Per neuron the batch
scan reduces to w_b = alpha*w_{b-1} + d_b (d = raw input current); the
spike-rate row is 0.1 at the first b with w_b >= THR (one-hot extracted via
prefix-max + shifted difference), else 0.

Implementation (241.8us baseline -> ~170us):
 - fp8e4m3 weights / counts / one-hot spikes + DoubleRow matmuls (K=256 per
   MM): GEMM1 is 128 MMs, GEMM2 32 MMs.  counts<=10 and fT in {0,1} are
   exact in fp8e4m3; weight rounding is ~1e-4 relative on the currents while
   first-crossing margins are orders of magnitude.
 - spike-count encode (stage A) entirely on DVE (concurrent DVE+GpSimd SBUF
   traffic degrades both ~2x), sigmoid on ScalarE; replication hoisted so
   compares start as soon as each uniform tile lands.
 - counts packed two-per-byte (c0 + 16*c1) and exchanged with two staged
   AllGathers (partition-major layout -> one contiguous DMA per rank on the
   way back; unpack via int8-rounding on ScalarE + one DVE op, scheduled in
   the PE-only chunk-A window).
 - GEMM1 chunk A (6 super-k, r-major) overlaps the second AllGather; chunk B
   is m-major with two held-back super-k per m-group, explicitly
   dependency-chained (the greedy scheduler otherwise transposes it to
   r-major and starves the scan pipeline).  Per-m first-crossing chains
   (PSUM-direct w-scan, prefix-max scan, gate + diff) run on DVE while PE
   continues; a GEMM2 wavefront fills PE gaps as psum banks retire.
 - output currents pre-scaled 1/64 into fp8, combined with AllToAll + local
   strided tree-sum (cheaper than ReduceScatter at this size), then the same
   first-crossing scan on the local 128-row output slice.
Cross-core skew + NRT barrier (~35-100us, run-variable) plus the
first-collective setup (~11us) dominate the remaining front of the kernel.
"""

import sys
import numpy as np

for _p in ("/opt/trn_rl_repo", "/root/.axon_site/_ro/trn_rl_repo"):
    if _p not in sys.path:
        sys.path.insert(0, _p)

import concourse.bass as bass
import concourse.mybir as mybir
import concourse.tile as tile
from concourse.tile_rust import add_dep_helper
from concourse import bacc
from concourse.bass_utils import run_bass_kernel_spmd

F32 = mybir.dt.float32
BF16 = mybir.dt.bfloat16
F8 = mybir.dt.float8e4
AL = mybir.AluOpType
ACT = mybir.ActivationFunctionType
DR = mybir.MatmulPerfMode.DoubleRow

B = 512            # batch (free dim everywhere)
IN_DIM = 4096
HID = 8192
OUT = 1024
T = 10
NCORES = 8
IN_SL = IN_DIM // NCORES    # 512 input dims per core
H_SL = HID // NCORES        # 1024 hidden per core
O_SL = OUT // NCORES        # 128 outputs per core
P = 128
SK = IN_DIM // 256          # 16 super-k (256-row) tiles for GEMM1
SM = H_SL // 256            # 4 super-m tiles for GEMM2

_LAM = np.float64(np.float32(0.95))
ALPHA = float(_LAM ** 10)                                # per-item decay
_G10 = float((1.0 - _LAM ** 10) / (1.0 - _LAM))          # per-item current gain
THR = float(10.0 / _G10)   # raw-current scan threshold (v>=1 <=> w>=THR)
RS_SCALE = 1.0 / 64.0      # pre-scale for fp8 output-current payload


def _build_nc():
    nc = bacc.Bacc(num_devices=NCORES)

    xt = nc.declare_dram_parameter("xt", [IN_SL, B], BF16, isOutput=False)
    u = nc.declare_dram_parameter("u", [IN_SL // P, P, T, B], BF16, isOutput=False)
    # w_ih[p, 2j+s, h] = W_ih[256j + 128s + p, hslice];  DoubleRow pairing
    w_ih = nc.declare_dram_parameter("w_ih", [P, 2 * SK, H_SL], F8, isOutput=False)
    # w_ho[p, 2j+s, o] = W_ho[hslice][256j + 128s + p, :]
    w_ho = nc.declare_dram_parameter("w_ho", [P, 2 * SM, OUT], F8, isOutput=False)
    res = nc.declare_dram_parameter("res", [O_SL, B], F32, isOutput=True)

    with tile.TileContext(nc, num_cores=NCORES) as tc:
        with (
            tc.tile_pool(name="const", bufs=1) as constp,
            tc.tile_pool(name="dram", bufs=1, space="DRAM") as dramp,
            tc.tile_pool(name="wih", bufs=1) as wpool,
            tc.tile_pool(name="who", bufs=1) as wopool,
            tc.tile_pool(name="stgA", bufs=2) as apool,
            tc.tile_pool(name="ubuf", bufs=2) as upool,
            tc.tile_pool(name="rep", bufs=2) as reppool,
            tc.tile_pool(name="cnt", bufs=2) as cntpool,
            tc.tile_pool(name="cp", bufs=8) as cppool,
            tc.tile_pool(name="scan", bufs=3) as spool,
            tc.tile_pool(name="fT", bufs=4) as fpool,
            tc.tile_pool(name="outb", bufs=4) as obpool,
            tc.tile_pool(name="fin", bufs=1) as finp,
        ):
            # ---- constants ----
            alpha_bf = constp.tile([P, B], BF16, name="alpha_bf")
            nc.vector.memset(alpha_bf, ALPHA)

            # ---- stage A: spike-count encode on the local input slice ----
            # counts land partition-major: cnt_localX[p, t*B + b] halves A/B.
            cnt_localA = dramp.tile([P, B], mybir.dt.uint8, name="cnt_localA")
            cnt_localB = dramp.tile([P, B], mybir.dt.uint8, name="cnt_localB")
            cnt_allA = dramp.tile([NCORES * P, B], mybir.dt.uint8, name="cnt_allA",
                                  addr_space="Shared")
            cnt_allB = dramp.tile([NCORES * P, B], mybir.dt.uint8, name="cnt_allB",
                                  addr_space="Shared")
            # hoist all input DMAs (sync queue is in-order; weights come later)
            xt_sbs, u_sbs = [], []
            for p in range(IN_SL // P):
                xt_sb = apool.tile([P, B], BF16, name="xt_sb", tag="xt", bufs=4)
                nc.sync.dma_start(xt_sb, xt[p * P:(p + 1) * P, :])
                xt_sbs.append(xt_sb)
            for p in range(IN_SL // P):
                u_sb = upool.tile([P, T * B], BF16, name="u_sb", tag="u", bufs=4)
                nc.sync.dma_start(u_sb, u[p].rearrange("p t b -> p (t b)"))
                u_sbs.append(u_sb)
            # sigmoids + replication first (no dependence on u loads)
            reps = []
            for p in range(IN_SL // P):
                sig = apool.tile([P, B], BF16, name="sig", tag="sig")
                nc.scalar.activation(sig, xt_sbs[p], ACT.Sigmoid)
                rep = reppool.tile([P, T * B], BF16, name="rep", tag="rep")
                nc.vector.tensor_copy(rep[:, 0:B], sig)
                nc.vector.tensor_copy(rep[:, B:2 * B], rep[:, 0:B])
                nc.vector.tensor_copy(rep[:, 2 * B:4 * B], rep[:, 0:2 * B])
                nc.vector.tensor_copy(rep[:, 4 * B:8 * B], rep[:, 0:4 * B])
                nc.vector.tensor_copy(rep[:, 8 * B:10 * B], rep[:, 0:2 * B])
                reps.append(rep)
            # compare + tree-sum as u tiles land (all on DVE: concurrent
            # DVE+GpSimd SBUF traffic degrades both ~2x)
            cnts = []
            for p in range(IN_SL // P):
                cmp = upool.tile([P, T * B], BF16, name="cmp", tag="cmp", bufs=1)
                nc.vector.tensor_tensor(cmp, u_sbs[p], reps[p], AL.is_lt)
                s1 = apool.tile([P, 5 * B], BF16, name="s1", tag="s1", bufs=1)
                nc.vector.tensor_tensor(s1, cmp[:, :5 * B], cmp[:, 5 * B:], AL.add)
                s2 = apool.tile([P, 2 * B], BF16, name="s2", tag="s2")
                nc.vector.tensor_tensor(s2, s1[:, :2 * B], s1[:, 2 * B:4 * B], AL.add)
                s3 = apool.tile([P, B], BF16, name="s3", tag="s3")
                nc.vector.tensor_tensor(s3, s2[:, :B], s2[:, B:], AL.add)
                cnt8 = cntpool.tile([P, B], BF16, name="cnt8", tag="cnt8")
                nc.vector.tensor_tensor(cnt8, s3, s1[:, 4 * B:], AL.add)
                cnts.append(cnt8)
                # pack two count tiles into one u8 plane: pk = c0 + 16*c1
                if p % 2 == 1:
                    pk = cntpool.tile([P, B], mybir.dt.uint8, name="pk", tag="pk")
                    nc.vector.scalar_tensor_tensor(
                        pk, cnts[p], 16.0, cnts[p - 1], AL.mult, AL.add)
                    dst = cnt_localA if p < 2 else cnt_localB
                    nc.gpsimd.dma_start(dst[:, :], pk)
                    nc.gpsimd.collective_compute(
                        "AllGather", AL.bypass,
                        replica_groups=[list(range(NCORES))],
                        ins=[(cnt_localA if p < 2 else cnt_localB)[:, :]],
                        outs=[(cnt_allA if p < 2 else cnt_allB)[:, :]],
                    )

            # ---- weight loads (after stage-A DMAs; single large DMAs) ----
            w_sb = wpool.tile([P, 2 * SK, H_SL], F8, name="w_sb")
            nc.sync.dma_start(
                w_sb.rearrange("p j h -> p (j h)"),
                w_ih.rearrange("p j h -> p (j h)"),
            )
            who_sb = wopool.tile([P, 2 * SM, OUT], F8, name="who_sb")
            nc.sync.dma_start(
                who_sb.rearrange("p j o -> p (j o)"),
                w_ho.rearrange("p j o -> p (j o)"),
            )

            with tc.tile_pool(name="ps", bufs=8, space="PSUM") as psp:
                psum_h = [psp.tile([P, B], F32, name=f"ph{m}", tag="ph")
                          for m in range(H_SL // P)]

                # ---- GEMM1 chunk A (r-major, super-k 2r; overlaps AG-B) ----
                def load_unpack(r, cnt_allX, tag, bufs):
                    pk = cppool.tile([P, B], mybir.dt.uint8, name=f"pk{tag}{r}",
                                     tag=f"pk{tag}", bufs=3)
                    nc.sync.dma_start(pk, cnt_allX[r * P:(r + 1) * P, :])
                    cp = cppool.tile([P, 2, B], F8, name=f"cp{tag}{r}",
                                     tag=f"cp{tag}", bufs=bufs)
                    cpf = cp.rearrange("p s b -> p (s b)")
                    # c1 ~= RNE_f8(pk/16 - 5/16) on ScalarE, c0 = pk - 16*c1 on
                    # DVE.  The f8 rounding is exact to the integer for c1>=4;
                    # below that c1 keeps a bounded fraction (|err|<=0.31) that
                    # c0 absorbs with the opposite sign, so the reconstructed
                    # current error per element is zero-mean and ~1e-4 relative
                    # after the 4096-wide contraction — harmless against the
                    # first-crossing margins (verified bit-exact on HW).
                    nc.scalar.activation(cpf[:, B:2 * B], pk, ACT.Copy,
                                         scale=1.0 / 16.0, bias=-5.0 / 16.0)
                    nc.vector.scalar_tensor_tensor(
                        cpf[:, 0:B], cpf[:, B:2 * B], -16.0, pk, AL.mult, AL.add)
                    return cp

                HOLD = 2      # super-k held back to feed PE during chunk B
                cpa_held = []
                for r in range(NCORES):
                    cp = load_unpack(r, cnt_allA, "a", 3)
                    if r >= NCORES - HOLD:
                        cpa_held.append((2 * r, cp))
                        continue
                    j = 2 * r
                    for m in range(H_SL // P):
                        nc.tensor.matmul(
                            psum_h[m],
                            lhsT=w_sb[:, 2 * j:2 * j + 2, m * P:(m + 1) * P],
                            rhs=cp[:, :, :],
                            start=(r == 0), stop=False,
                            perf_mode=DR,
                        )

                # ---- GEMM1 chunk B (m-major, super-k 2r+1) + pipelined scans --
                cpb = []
                for r in range(NCORES):
                    cpb.append(load_unpack(r, cnt_allB, "b", 8))

                fT = []        # [P, 2, B] fp8 one-hot spike tiles per super-m
                a2a_in = dramp.tile([OUT, B], F8, name="a2a_in")
                psum_o = [psp.tile([P, B], F32, name=f"po{o}", tag="ph")
                          for o in range(OUT // P)]
                g2_emitted = set()

                def emit_gemm2(avail_o, max_j):
                    for jj in range(max_j + 1):
                        for o in range(avail_o):
                            if (jj, o) in g2_emitted:
                                continue
                            g2_emitted.add((jj, o))
                            nc.tensor.matmul(
                                psum_o[o],
                                lhsT=who_sb[:, 2 * jj:2 * jj + 2,
                                            o * P:(o + 1) * P],
                                rhs=fT[jj][:, :, :],
                                start=(jj == 0), stop=(jj == SM - 1),
                                perf_mode=DR,
                            )
                            if jj == SM - 1:
                                ob = obpool.tile([P, B], F8, name="ob", tag="ob")
                                if o % 2 == 0:
                                    nc.scalar.activation(ob, psum_o[o], ACT.Copy,
                                                         scale=RS_SCALE)
                                else:
                                    nc.vector.tensor_scalar(
                                        ob, psum_o[o], RS_SCALE, None, AL.mult)
                                (nc.sync if o % 2 == 0 else nc.scalar).dma_start(
                                    a2a_in[o * P:(o + 1) * P, :], ob)

                f8t = None
                pm_pair = None
                prev_last_mm = None
                for m in range(H_SL // P):
                    first_mm = None
                    for j, cp in cpa_held:
                        mm = nc.tensor.matmul(
                            psum_h[m],
                            lhsT=w_sb[:, 2 * j:2 * j + 2, m * P:(m + 1) * P],
                            rhs=cp[:, :, :],
                            start=False, stop=False,
                            perf_mode=DR,
                        )
                        first_mm = first_mm or mm
                    for r in range(NCORES):
                        j = 2 * r + 1
                        mm = nc.tensor.matmul(
                            psum_h[m],
                            lhsT=w_sb[:, 2 * j:2 * j + 2, m * P:(m + 1) * P],
                            rhs=cpb[r][:, :, :],
                            start=False, stop=(r == NCORES - 1),
                            perf_mode=DR,
                        )
                        first_mm = first_mm or mm
                    if prev_last_mm is not None:
                        # pin m-major order: the greedy scheduler otherwise
                        # transposes chunk B to r-major (cpb tiles become
                        # ready progressively), starving the scan pipeline
                        add_dep_helper(first_mm.ins, prev_last_mm.ins, sync=True,
                                       reason="chunk-B m-major order")
                    prev_last_mm = mm
                    # first-crossing chain for m-tile (overlaps later matmuls):
                    # scans on DVE (only engine with scan), gate+diff on GpSimd.
                    # the w-scan reads PSUM directly (same 1x speed, frees the
                    # bank on completion, no Scalar evac in the chain)
                    w_sc = spool.tile([P, B], BF16, name="w_sc", tag="wsc")
                    nc.vector.tensor_tensor_scan(
                        w_sc, alpha_bf, psum_h[m], 0.0, AL.mult, AL.add)
                    if m % 2 == 0:
                        pm_pair = spool.tile([P, 2 * B], BF16, name="pm", tag="pm")
                        f8t = fpool.tile([P, 2, B], F8, name=f"fT{m // 2}", tag="fT")
                    nc.vector.tensor_tensor_scan(
                        pm_pair[:, (m % 2) * B:(m % 2 + 1) * B],
                        w_sc, w_sc, 0.0, AL.max, AL.max)
                    if m % 2 == 1:
                        # gate + step-up diff on DVE: shortest fT chain; the
                        # GEMM2 wavefront keeps PE ahead of DVE throughput
                        f8f = f8t.rearrange("p s b -> p (s b)")
                        gp = spool.tile([P, 2 * B], BF16, name="gp", tag="gp")
                        nc.vector.tensor_scalar(gp, pm_pair, THR, None, AL.is_ge)
                        # diff on GpSimd (idle here): DVE is the pair-throughput
                        # binder during chunk B, so keep it to scans + gate only
                        nc.gpsimd.tensor_tensor(
                            f8f[:, 1:], gp[:, 1:], gp[:, :2 * B - 1], AL.subtract)
                        nc.scalar.copy(f8f[:, 0:1], gp[:, 0:1])
                        nc.scalar.copy(f8f[:, B:B + 1], gp[:, B:B + 1])
                        fT.append(f8t)
                        # GEMM2 wavefront: emit every (j, o) whose fT and psum
                        # bank are now available, filling PE's scan-wait gaps
                        emit_gemm2(avail_o=min(2 * (m // 2 + 1), OUT // P),
                                   max_j=m // 2)

            # ---- AllToAll: shard o-slices across cores; local tree-sum ----
            a2a_out = dramp.tile([OUT, B], F8, name="a2a_out")
            nc.gpsimd.collective_compute(
                "AllToAll", AL.bypass,
                replica_groups=[list(range(NCORES))],
                ins=[a2a_in[:, :]], outs=[a2a_out[:, :]],
            )
            # load the 8 partial tiles into one [P, 8, B] buffer (issue spread
            # across the three DMA-capable queues), then 3 strided tree adds
            pbuf = finp.tile([P, NCORES, B], F8, name="pbuf")
            pflat = pbuf.rearrange("p i b -> p (i b)")
            engs = [nc.sync, nc.scalar]   # gpsimd queue pays a DRAIN here
            for i in range(NCORES):
                engs[i % 2].dma_start(pflat[:, i * B:(i + 1) * B],
                                      a2a_out[i * P:(i + 1) * P, :])
            q4 = finp.tile([P, 4, B], BF16, name="q4")
            nc.vector.tensor_tensor(q4[:, :, :], pbuf[:, 0:4, :],
                                    pbuf[:, 4:8, :], AL.add)
            q2 = finp.tile([P, 2, B], BF16, name="q2")
            nc.vector.tensor_tensor(q2[:, :, :], q4[:, 0:2, :],
                                    q4[:, 2:4, :], AL.add)
            cur_o = finp.tile([P, B], BF16, name="cur_o")
            nc.vector.tensor_tensor(cur_o, q2[:, 0, :], q2[:, 1, :], AL.add)

            # ---- output layer: same scan + first-crossing, scaled by 0.1 ----
            wo = spool.tile([P, B], BF16, name="wo", tag="wsc")
            nc.vector.tensor_tensor_scan(wo, alpha_bf, cur_o, 0.0, AL.mult, AL.add)
            pmo = spool.tile([P, B], BF16, name="pmo", tag="pm2")
            nc.vector.tensor_tensor_scan(pmo, wo, wo, 0.0, AL.max, AL.max)
            g2 = spool.tile([P, B], F32, name="g2", tag="g2")
            nc.vector.tensor_scalar(
                g2, pmo, THR * RS_SCALE, float(np.float32(0.1)),
                AL.is_ge, AL.mult)
            out_sb = spool.tile([P, B], F32, name="out_sb", tag="outsb")
            nc.vector.tensor_tensor(
                out_sb[:, 1:], g2[:, 1:], g2[:, :B - 1], AL.subtract)
            nc.vector.tensor_copy(out_sb[:, 0:1], g2[:, 0:1])
            nc.sync.dma_start(res[:, :], out_sb)

    nc.finalize()
    return nc


_STATE = {}


def _get_uniforms():
    """The key-42 uniform draws the reference's bernoulli uses — input-independent
    constants. [B, IN_DIM, T] float32, computed once on host."""
    if "u" not in _STATE:
        import jax
        import jax.numpy as jnp
        f = jax.jit(lambda: jax.random.uniform(
            jax.random.key(42), (B, IN_DIM, T), jnp.float32), backend="cpu")
        _STATE["u"] = np.asarray(f())
    return _STATE["u"]


def _get_nc():
    if "nc" not in _STATE:
        _STATE["nc"] = _build_nc()
    return _STATE["nc"]


def make_in_maps(x, W_ih, W_ho):
    import ml_dtypes

    F8NP = ml_dtypes.float8_e4m3
    x = np.ascontiguousarray(x, dtype=np.float32)
    W_ih = np.ascontiguousarray(W_ih, dtype=np.float32)
    W_ho = np.ascontiguousarray(W_ho, dtype=np.float32)
    u = _get_uniforms()

    in_maps = []
    for m in range(NCORES):
        isl = slice(m * IN_SL, (m + 1) * IN_SL)
        # u[b, i, t] -> [i_slice, t, b] -> [4, 128, T, B] bf16
        uc = np.ascontiguousarray(
            u[:, isl, :].transpose(1, 2, 0).reshape(IN_SL // P, P, T, B)
        ).astype(ml_dtypes.bfloat16)
        # W_ih[:, hslice] -> [p, 2j+s, h] DoubleRow pairing, fp8
        wi = W_ih[:, m * H_SL:(m + 1) * H_SL].reshape(SK, 2, P, H_SL)
        wi = np.ascontiguousarray(wi.transpose(2, 0, 1, 3).reshape(P, 2 * SK, H_SL))
        # W_ho[hslice, :] -> [p, 2j+s, o], fp8
        wo = W_ho[m * H_SL:(m + 1) * H_SL, :].reshape(SM, 2, P, OUT)
        wo = np.ascontiguousarray(wo.transpose(2, 0, 1, 3).reshape(P, 2 * SM, OUT))
        in_maps.append({
            "xt": np.ascontiguousarray(x[:, isl].T).astype(ml_dtypes.bfloat16),
            "u": uc,
            "w_ih": wi.astype(F8NP),
            "w_ho": wo.astype(F8NP),
        })
    return in_maps


def assemble_out(results):
    out = np.empty((B, OUT), np.float32)
    for m in range(NCORES):
        out[:, m * O_SL:(m + 1) * O_SL] = results[m]["res"].T
    return out


def kernel(x, W_ih, W_ho):
    nc = _get_nc()
    in_maps = make_in_maps(x, W_ih, W_ho)
    r = run_bass_kernel_spmd(nc, in_maps, list(range(NCORES)))

    return assemble_out(r.results)


if __name__ == "__main__":
    rng = np.random.default_rng(0)
    x = rng.standard_normal((B, IN_DIM), dtype=np.float32)
    W_ih = np.clip(0.5 + 0.1 * rng.standard_normal((IN_DIM, HID)), 0, 1).astype(np.float32)
    W_ho = np.clip(0.5 + 0.1 * rng.standard_normal((HID, OUT)), 0, 1).astype(np.float32)
    out = kernel(x, W_ih, W_ho)
    print("out", out.shape, out.dtype, "nonzero rows:", np.unique(np.nonzero(out)[0]))
